# revision 27
# baseline (speedup 1.0000x reference)
"""Bass/Trainium2 kernel for the BiLSTM tagger problem (transposed design).

Self-contained: builds an SPMD bass program (same program on all 8 cores,
data-parallel over the batch: 16 sentences/core), runs it via a bass2jax
shard_map runner, and gathers the full [128, 256, 50] output.

Layout: everything transposed — partition dim = feature dim, free dim =
(cell, h-chunk, batch).  Per core (Bl=16, T=256):

  E   : gather bf16 embeddings (indirect DMA, ends-inward tile order so L1
        can start immediately) + PE-transpose -> embT [128(E), 4096] bf16
  L1/L2 recurrence, fwd+bwd cells packed per step s (tf=s, tb=T-1-s):
        gatesT psum [128, 256] f32, col = gate(o,i,f,g)*64 + cc*32 + hc*16 + b
        bias  : 1 matmul  K=16 selector (WbT [16,128] @ S [16,256])
        pre   : L1: 16 mm N=16 (embT cols), L2: 64 mm N=16 (hist1 cols)
        hh    : 32 mm N=16 (lhsT = WhhT block [128,128], rhs = hT scratch)
        sgi   = sigmoid(gatesT[i,f,g])          (ACT; tanh via 2sig(2x)-1)
        sgo   = sigmoid(gatesT[o])              (ACT, off critical path)
        b     = sgi_f * c_prev                  (DVE tt)
        a'    = (sgi_g - 0.5) * sgi_i           (DVE stt)   [a = 2a']
        c     = 2*a' + b                        (DVE stt)
        tc'   = sigmoid(2c)                     (ACT, scale=2) [tanh(c)=2tc'-1]
        hT    = (tc' - 0.5) * sgo               (DVE stt -> scratch, feeds hh)
        hist_f/hist_b <- hT halves              (Pool copies, off the chain)
        hist stores h' = h/2; consumers (Whh, W2ih, wout) host-scaled x2.
  OUT : interleaved into the L2 loop as token tiles complete:
        per tile g: 8x(1+4) mm N=50 + Pool copies -> out [4096, 50] f32
"""

import os
import numpy as np
import ml_dtypes

B, T_FULL = 128, 256
VOCAB, EMB, HID, TAGS = 50000, 128, 256, 50
NCORES = 8
BL = B // NCORES            # 16 sentences per core
G4 = 4 * HID                # 1024
PSUM_BUFS = int(os.environ.get("K_PSUM_BUFS", "4"))
SG_BUFS = int(os.environ.get("K_SG_BUFS", "4"))


def _patched_tile_context(nc):
    """TileContext whose final drain splits sem waits across nops (this
    walrus build allows only one sync wait on control instructions)."""
    import concourse.tile as tile
    from concourse import mybir

    class PatchedTileContext(tile.TileContext):
        MAX_W = 1       # control insts (nop/drain) + PE (ldweights encoding)
        MAX_W_SOFT = int(os.environ.get("K_MAXW", "1"))  # other engines

        def _add_instruction(self, inst):
            si = inst.sync_info
            lim = self.MAX_W
            if inst.engine in (mybir.EngineType.PE, mybir.EngineType.SP):
                lim = self.MAX_W
            elif not isinstance(inst, (mybir.InstNoOp, mybir.InstDrain)):
                lim = self.MAX_W_SOFT
            if si is not None and si.on_wait and len(si.on_wait) > lim:
                waits = list(si.on_wait)
                si.on_wait = waits[-lim:]
                rest = waits[:-lim]
                while rest:
                    nop = mybir.InstNoOp(
                        name=self.nc.get_next_instruction_name(),
                        ins=[], outs=[])
                    nop.engine = inst.engine
                    nop.sync_info = mybir.SyncInfo(
                        on_wait=rest[:self.MAX_W], on_update=[])
                    rest = rest[self.MAX_W:]
                    super()._add_instruction(nop)
            super()._add_instruction(inst)

        def _drain_and_barrier(self, tick_clock, wait_clock):
            nop_inst = self.nc.sync.nop()
            wait_clock.add_sem_waits(
                nop_inst.ins, tile.ScopedClock({None: tick_clock.global_clock})
            )
            si = nop_inst.ins.sync_info
            waits = list(si.on_wait) if si is not None else []
            MAX_W = 1
            if len(waits) > MAX_W:
                si.on_wait = waits[:MAX_W]
                rest = waits[MAX_W:]
                while rest:
                    extra = self.nc.sync.nop()
                    extra.ins.sync_info = mybir.SyncInfo(
                        on_wait=rest[:MAX_W], on_update=[]
                    )
                    rest = rest[MAX_W:]
            self.nc.sync.drain()
            self.nc.all_engine_barrier()
            assert self.sems is not None
            popped = self.nc._tile_sem_poison_stack.pop()
            assert popped is self._sem_poison
            self.nc.clear_and_free_semaphores(list(self.sems.allocated().values()))
            self.nc.all_engine_barrier()

    return PatchedTileContext(nc)


def build_program(T=T_FULL):
    import concourse.bass as bass
    import concourse.mybir as mybir

    f32 = mybir.dt.float32
    i32 = mybir.dt.int32
    bf16 = mybir.dt.bfloat16
    SIG = mybir.ActivationFunctionType.Sigmoid
    MUL = mybir.AluOpType.mult
    ADD = mybir.AluOpType.add
    SUB = mybir.AluOpType.subtract

    NTOK = BL * T
    NTT = NTOK // 128       # token tiles (32)

    nc = bass.Bass()

    # ---------------- I/O ----------------
    sent = nc.dram_tensor("sent", [128, NTT], i32, kind="ExternalInput")
    emb_d = nc.dram_tensor("emb", [VOCAB, EMB], bf16, kind="ExternalInput")
    ident_d = nc.dram_tensor("ident128b", [128, 128], bf16, kind="ExternalInput")
    onescol_d = nc.dram_tensor("onescol", [1, 128], bf16, kind="ExternalInput")
    bsel_d = nc.dram_tensor("bsel", [16, 512], bf16, kind="ExternalInput")
    w_in = {}
    for cell in ("1f", "1b"):
        w_in[f"wih{cell}"] = nc.dram_tensor(f"wih{cell}", [EMB, G4], bf16,
                                            kind="ExternalInput")
    for cell in ("2f", "2b"):
        w_in[f"wih{cell}"] = nc.dram_tensor(f"wih{cell}", [2 * HID, G4], bf16,
                                            kind="ExternalInput")
    for cell in ("1f", "1b", "2f", "2b"):
        w_in[f"whh{cell}"] = nc.dram_tensor(f"whh{cell}", [HID, G4], bf16,
                                            kind="ExternalInput")
    wb_d = {1: nc.dram_tensor("wb1", [16, 128], bf16, kind="ExternalInput"),
            2: nc.dram_tensor("wb2", [16, 128], bf16, kind="ExternalInput")}
    wout_d = nc.dram_tensor("woutT", [2 * HID, TAGS], bf16, kind="ExternalInput")
    bout_d = nc.dram_tensor("bout", [1, TAGS], bf16, kind="ExternalInput")
    out_d = nc.dram_tensor("out", [TAGS, NTOK], f32, kind="ExternalOutput")

    tc = _patched_tile_context(nc)
    with tc:
        with tc.tile_pool(name="const", bufs=1) as cp:
            ident = cp.tile([128, 128], bf16)
            nc.sync.dma_start(ident[:], ident_d[:])
            onescol = cp.tile([1, 128], bf16)
            nc.sync.dma_start(onescol[:], onescol_d[:])
            bsel = cp.tile([16, 512], bf16)
            nc.sync.dma_start(bsel[:], bsel_d[:])
            NTT_ = BL * T // 128
            sidx = cp.tile([128, NTT_], i32, name="sidx")
            nc.sync.dma_start(sidx[:], sent[:, 0:NTT_])
            wb = {}
            for layer in (1, 2):
                wb[layer] = cp.tile([16, 128], bf16, tag=f"wb{layer}",
                                    name=f"swb{layer}")
                nc.sync.dma_start(wb[layer][:], wb_d[layer][:])
            # --- all LSTM weights, preloaded upfront; L1's first so its
            # first steps aren't stuck behind the (bigger) L2 loads ---
            wih1, whh = {}, {}
            for i, cell in enumerate(("1f", "1b")):
                wt = cp.tile([EMB, G4], bf16, tag=f"wih{cell}",
                             name=f"swih{cell}")
                nc.sync.dma_start(wt[:], w_in[f"wih{cell}"][:])
                wih1[i] = wt
            for cell in ("1f", "1b", "2f", "2b"):
                hh = []
                for k in range(2):
                    ht = cp.tile([128, G4], bf16, tag=f"whh{cell}{k}",
                                 name=f"swhh{cell}{k}")
                    nc.sync.dma_start(
                        ht[:], w_in[f"whh{cell}"][128 * k:128 * (k + 1), :])
                    hh.append(ht)
                whh[cell] = hh
            wih2 = {}
            for i, cell in enumerate(("2f", "2b")):
                ch = []
                for k in range(4):
                    wt = cp.tile([128, G4], bf16, tag=f"wih{cell}{k}",
                                 name=f"swih{cell}{k}")
                    nc.sync.dma_start(
                        wt[:], w_in[f"wih{cell}"][128 * k:128 * (k + 1), :])
                    ch.append(wt)
                wih2[i] = ch
            wout_ch = []
            for k in range(4):
                wt = cp.tile([128, TAGS], bf16, tag=f"wout{k}", name=f"swout{k}")
                nc.sync.dma_start(wt[:], wout_d[128 * k:128 * (k + 1), :])
                wout_ch.append(wt)
            bout = cp.tile([1, TAGS], bf16)
            nc.sync.dma_start(bout[:], bout_d[:])
            whh1 = {0: whh["1f"], 1: whh["1b"]}
            whh2 = {0: whh["2f"], 1: whh["2b"]}

            def lstm_layer(tc, layer, pre_pair, whhl, histf, histb, T,
                           post_step=None, pre_step=None, rev_copy=None):
                """T steps (as T/2 two-step pairs), f+b cells packed.
                pre_pair(gpr, sp, cc, gc) issues the input-projection matmuls
                for one gate-chunk as [2-step, 16] blocks into the pair tile.
                Gate col layout per step: gate(o,i,f,g)*64 + cc*32 + hc*16 + b.
                h' is written straight into histf/histb (no scratch: slices
                are written once, so the writes carry no WAR waits);
                rev_copy(cc, t, s) mirrors them into reversed-order tiles."""
                with tc.tile_pool(name=f"l{layer}_work", bufs=SG_BUFS) as lp, \
                        tc.tile_pool(name=f"l{layer}_cp", bufs=3) as cpp, \
                        tc.tile_pool(name=f"l{layer}_psum", bufs=PSUM_BUFS,
                                     space="PSUM") as pp:
                    c_prev = None
                    hist = {0: histf, 1: histb}
                    for sp in range(T // 2):
                        gpp = pp.tile([128, 512], f32, tag="gp", name="gp")
                        gpr = gpp.rearrange('p (t x) -> p t x', t=2)
                        # bias for both steps (K=16 selector matmul)
                        nc.tensor.matmul(gpp[:], wb[layer][:], bsel[:],
                                         start=True, stop=False,
                                         skip_group_check=True)
                        # input projections for the pair
                        for cc in (0, 1):
                            for gc in range(8):
                                pre_pair(gpr, sp, cc, gc)
                        for shalf in range(2):
                            s = 2 * sp + shalf
                            if pre_step is not None:
                                pre_step(s)
                            ts_ = {0: s, 1: T - 1 - s}
                            gp = gpp[:, shalf * 256:shalf * 256 + 256]
                            # hh: f-cell block first (h'_f lands before h'_b)
                            if s > 0:
                                for cc in (0, 1):
                                    tp = ts_[cc] + (1 if cc else -1)
                                    for gc in range(8):
                                        gt, hc = gc // 2, gc % 2
                                        cbase = gt * 64 + cc * 32 + hc * 16
                                        csl = slice(cbase, cbase + 16)
                                        for kc in range(2):
                                            nc.tensor.matmul(
                                                gp[:, csl],
                                                whhl[cc][kc][:, gc * 128:
                                                             (gc + 1) * 128],
                                                hist[cc][:, tp * 32 + kc * 16:
                                                          tp * 32 + kc * 16 + 16],
                                                start=False, stop=(kc == 1),
                                                skip_group_check=True)
                            # ---- pointwise ----
                            sgi = lp.tile([128, 192], f32, tag="sgi", name="sgi")
                            nc.scalar.activation(sgi[:], gp[:, 64:256], SIG)
                            sgo = lp.tile([128, 64], f32, tag="sgo", name="sgo")
                            nc.scalar.activation(sgo[:], gp[:, 0:64], SIG)
                            ap_t = lp.tile([128, 64], f32, tag="ap")
                            if s == 0:
                                nc.vector.scalar_tensor_tensor(
                                    ap_t[:], sgi[:, 128:192], 0.5, sgi[:, 0:64],
                                    SUB, MUL)
                                c_new = cpp.tile([128, 64], f32, tag="c",
                                                 name="c")
                                nc.vector.tensor_scalar(c_new[:], ap_t[:], 2.0,
                                                        None, MUL)
                            else:
                                b_t = lp.tile([128, 64], f32, tag="bb")
                                nc.vector.tensor_tensor(
                                    b_t[:], sgi[:, 64:128], c_prev[:], MUL)
                                nc.vector.scalar_tensor_tensor(
                                    ap_t[:], sgi[:, 128:192], 0.5, sgi[:, 0:64],
                                    SUB, MUL)
                                c_new = cpp.tile([128, 64], f32, tag="c",
                                                 name="c")
                                nc.vector.scalar_tensor_tensor(
                                    c_new[:], ap_t[:], 2.0, b_t[:], MUL, ADD)
                            c_prev = c_new
                            tcp = lp.tile([128, 64], f32, tag="tc")
                            nc.scalar.activation(tcp[:], c_new[:], SIG,
                                                 scale=2.0)
                            for cc in (0, 1):
                                t = ts_[cc]
                                nc.vector.scalar_tensor_tensor(
                                    hist[cc][:, t * 32:t * 32 + 32],
                                    tcp[:, cc * 32:cc * 32 + 32], 0.5,
                                    sgo[:, cc * 32:cc * 32 + 32], SUB, MUL)
                            if rev_copy is not None:
                                for cc in (0, 1):
                                    rev_copy(cc, ts_[cc])
                            if post_step is not None:
                                post_step(s)

            # ================= hist + embT =================
            with tc.tile_pool(name="hist", bufs=1) as hp:
                h1f = hp.tile([128, 32 * T], bf16, tag="h1f", name="h1f")
                h1b = hp.tile([128, 32 * T], bf16, tag="h1b", name="h1b")
                # reversed-order mirrors (col s*32 <-> token T-1-s) so the
                # L2 b-cell's pair-blocked projections read ascending slices
                h1fr = hp.tile([128, 32 * T], bf16, tag="h1fr", name="h1fr")
                h1br = hp.tile([128, 32 * T], bf16, tag="h1br", name="h1br")
                h2f = hp.tile([128, 32 * T], bf16, tag="h2f", name="h2f")
                h2b = hp.tile([128, 32 * T], bf16, tag="h2b", name="h2b")

                with tc.tile_pool(name="l1_fix", bufs=1) as p_l1:
                    embT = p_l1.tile([128, NTOK], bf16)
                    embTr = p_l1.tile([128, NTOK], bf16, name="embTr")
                    # gather ends-inward, interleaved with the L1 steps, so
                    # L1 (f from tile 0, b from tile 31) starts immediately
                    # and the middle tiles stream in ahead of consumption
                    with tc.tile_pool(name="embp", bufs=4) as ep, \
                            tc.tile_pool(name="embpp", bufs=2,
                                         space="PSUM") as epp:
                        def gather(g):
                            et = ep.tile([128, EMB], bf16, tag="et")
                            nc.gpsimd.indirect_dma_start(
                                out=et[:], out_offset=None,
                                in_=emb_d[:],
                                in_offset=bass.IndirectOffsetOnAxis(
                                    ap=sidx[:, g:g + 1], axis=0),
                            )
                            etp = epp.tile([128, EMB], bf16, tag="etp")
                            nc.tensor.transpose(etp[:], et[:], ident[:])
                            nc.vector.tensor_copy(
                                embT[:, 128 * g:128 * (g + 1)], etp[:])
                            # mirrored copy: token t -> col (T-1-t)*16
                            # (from embT: gpsimd cannot read PSUM)
                            for tau in range(8):
                                t = g * 8 + tau
                                nc.gpsimd.tensor_copy(
                                    embTr[:, (T - 1 - t) * 16:
                                          (T - 1 - t) * 16 + 16],
                                    embT[:, t * 16:t * 16 + 16])

                        for g in (0, NTT - 1, 1, NTT - 2):
                            gather(g)

                        def pre_step1(s):
                            if s % 8 == 0 and s // 8 + 2 <= NTT // 2 - 1:
                                k = s // 8
                                gather(k + 2)
                                gather(NTT - 3 - k)

                        def pre1(gpr, sp, cc, gc):
                            gt, hc = gc // 2, gc % 2
                            off = gt * 64 + cc * 32 + hc * 16
                            src = embT if cc == 0 else embTr
                            nc.tensor.matmul(
                                gpr[:, :, off:off + 16],
                                wih1[cc][:, gc * 128:(gc + 1) * 128],
                                src[:, sp * 32:sp * 32 + 32],
                                start=False, stop=False,
                                skip_group_check=True)

                        def rev1(cc, t):
                            src = h1f if cc == 0 else h1b
                            dst = h1fr if cc == 0 else h1br
                            nc.gpsimd.tensor_copy(
                                dst[:, (T - 1 - t) * 32:(T - 1 - t) * 32 + 32],
                                src[:, t * 32:t * 32 + 32])

                        # inside the gather pools: no drain barrier, so L1
                        # starts as soon as the edge tiles land
                        lstm_layer(tc, 1, pre1, whh1, h1f, h1b, T,
                                   pre_step=pre_step1, rev_copy=rev1)
                # p_l1 closed: embT freed

                # ================= L2 (+ interleaved OUT) =================
                hist1 = {0: h1f, 1: h1b}
                hist1r = {0: h1fr, 1: h1br}
                hist2 = {0: h2f, 1: h2b}
                h1fv = {0: h1f.rearrange('p (t x) -> p t x', t=T),
                        1: h1b.rearrange('p (t x) -> p t x', t=T)}
                h1rv = {0: h1fr.rearrange('p (t x) -> p t x', t=T),
                        1: h1br.rearrange('p (t x) -> p t x', t=T)}

                def pre2(gpr, sp, cc, gc):
                    gt, hc = gc // 2, gc % 2
                    off = gt * 64 + cc * 32 + hc * 16
                    srcs = h1fv if cc == 0 else h1rv
                    for kc in range(4):
                        src = srcs[kc // 2]
                        ko = (kc % 2) * 16
                        nc.tensor.matmul(
                            gpr[:, :, off:off + 16],
                            wih2[cc][kc][:, gc * 128:(gc + 1) * 128],
                            src[:, 2 * sp:2 * sp + 2, ko:ko + 16],
                            start=False, stop=False, skip_group_check=True)

                # OUT tile g is ready after L2 step max(8g+7, 255-8g)
                ready = {}
                for g in range(NTT):
                    ready.setdefault(max(8 * g + 7, T - 1 - 8 * g), []).append(g)

                with tc.tile_pool(name="outw", bufs=3) as ow, \
                        tc.tile_pool(name="outp", bufs=2, space="PSUM") as op:

                    def out_tile(g):
                        # transposed: ps[tag, token] so per-step results land
                        # in free-dim column slices (no partition-base limits)
                        ps = op.tile([TAGS, 128], f32, tag="ops")
                        nc.tensor.matmul(ps[:], bout[:1, :], onescol[:1, :],
                                         start=True, stop=False,
                                         skip_group_check=True)
                        for tau in range(8):
                            t = g * 8 + tau
                            for k in range(4):
                                cc, hc = k // 2, k % 2
                                nc.tensor.matmul(
                                    ps[:, tau * 16:tau * 16 + 16],
                                    wout_ch[k][:],
                                    hist2[cc][:, t * 32 + hc * 16:
                                              t * 32 + hc * 16 + 16],
                                    start=False, stop=(k == 3),
                                    skip_group_check=True)
                        sb = ow.tile([TAGS, 128], f32, tag="osb")
                        nc.vector.tensor_copy(sb[:], ps[:])
                        nc.sync.dma_start(out_d[:, 128 * g:128 * (g + 1)],
                                          sb[:])

                    def post_step(s):
                        for g in ready.get(s, ()):
                            out_tile(g)

                    lstm_layer(tc, 2, pre2, whh2, h2f, h2b, T,
                               post_step=post_step)

    return nc


def _prep_cell_weights(wih, whh, bih, bhh, h_in_scale):
    """Gate perm i,f,g,o -> o,i,f,g; scale g rows x2 (tanh(x)=2*sig(2x)-1);
    whh rows x2 and wih rows x h_in_scale compensate the h'=h/2 storage.
    Returns (wihT, whhT, brow) f32 with transposed [Din, 4H] layout."""
    H = HID
    idx = np.concatenate([np.arange(3 * H, 4 * H),      # o
                          np.arange(0, H),              # i
                          np.arange(H, 2 * H),          # f
                          np.arange(2 * H, 3 * H)])     # g
    gscale = np.ones((4 * H, 1), np.float32)
    gscale[3 * H:4 * H] = 2.0
    wih_p = wih[idx] * gscale * h_in_scale
    whh_p = whh[idx] * gscale * 2.0
    b_p = (bih + bhh)[idx] * gscale[:, 0]
    return (np.ascontiguousarray(wih_p.T, np.float32),
            np.ascontiguousarray(whh_p.T, np.float32),
            np.ascontiguousarray(b_p, np.float32))


class Runner:
    """Build the SPMD program once; execute repeatedly on device-resident
    inputs (for clean timing, no donation so buffers are reusable)."""

    def __init__(self, nc, n_cores=NCORES):
        import jax
        import numpy as _np
        from jax.sharding import Mesh, PartitionSpec
        from jax.experimental.shard_map import shard_map
        import concourse.mybir as mybir
        from concourse import bass2jax as b2j

        b2j.install_neuronx_cc_hook()
        self.jax = jax
        self.nc = nc
        self.n_cores = n_cores
        partition_name = (nc.partition_id_tensor.name
                          if nc.partition_id_tensor else None)
        in_names, out_names, out_avals, zero_outs = [], [], [], []
        for alloc in nc.m.functions[0].allocations:
            if not isinstance(alloc, mybir.MemoryLocationSet):
                continue
            name = alloc.memorylocations[0].name
            if alloc.kind == "ExternalInput":
                if name != partition_name:
                    in_names.append(name)
            elif alloc.kind == "ExternalOutput":
                out_names.append(name)
                shape = tuple(alloc.tensor_shape)
                dtype = mybir.dt.np(alloc.dtype)
                out_avals.append(jax.core.ShapedArray(shape, dtype))
                zero_outs.append(_np.zeros(shape, dtype))
        self.n_params = len(in_names)
        self.in_names = list(in_names)
        self.out_names = list(out_names)
        self.out_avals = out_avals
        self.zero_outs = zero_outs
        all_in = in_names + out_names
        if partition_name is not None:
            all_in.append(partition_name)

        def _body(*args):
            operands = list(args)
            if partition_name is not None:
                operands.append(b2j.partition_id_tensor())
            outs = b2j._bass_exec_p.bind(
                *operands,
                out_avals=tuple(out_avals),
                in_names=tuple(all_in),
                out_names=tuple(out_names),
                lowering_input_output_aliases=(),
                sim_require_finite=True,
                sim_require_nnan=True,
                nc=nc,
            )
            return tuple(outs)

        devices = jax.devices()[:n_cores]
        self.mesh = Mesh(_np.asarray(devices), ("core",))
        in_specs = (PartitionSpec("core"),) * (self.n_params + len(out_names))
        out_specs = (PartitionSpec("core"),) * len(out_names)
        self.sharded = jax.jit(shard_map(_body, mesh=self.mesh,
                                         in_specs=in_specs,
                                         out_specs=out_specs, check_rep=False),
                               keep_unused=True)
        self.dev_args = None

    def put(self, in_maps):
        """Upload per-core input maps as device-sharded global arrays."""
        import numpy as _np
        from jax.sharding import NamedSharding, PartitionSpec
        jax = self.jax
        sh = NamedSharding(self.mesh, PartitionSpec("core"))
        args = []
        for name in self.in_names:
            g = _np.concatenate([_np.asarray(m[name]) for m in in_maps], axis=0)
            args.append(jax.device_put(g, sh))
        for z in self.zero_outs:
            g = _np.zeros((self.n_cores * z.shape[0],) + z.shape[1:], z.dtype)
            args.append(jax.device_put(g, sh))
        self.dev_args = args

    def run(self):
        outs = self.sharded(*self.dev_args)
        self.jax.block_until_ready(outs)
        return outs

    def results(self, outs):
        import numpy as _np
        res = []
        for c in range(self.n_cores):
            res.append({name: _np.asarray(outs[i]).reshape(
                (self.n_cores,) + self.out_avals[i].shape)[c]
                for i, name in enumerate(self.out_names)})
        return res

    def time_exec(self, iters=10):
        import time as _time
        self.run()  # warm
        best = float("inf")
        for _ in range(iters):
            t0 = _time.perf_counter()
            self.run()
            best = min(best, _time.perf_counter() - t0)
        return best


_RUNNERS = {}


def get_runner(T=T_FULL):
    if T not in _RUNNERS:
        _RUNNERS[T] = Runner(build_program(T))
    return _RUNNERS[T]


def make_in_maps(sentence, emb,
                 wih1f, whh1f, bih1f, bhh1f,
                 wih1b, whh1b, bih1b, bhh1b,
                 wih2f, whh2f, bih2f, bhh2f,
                 wih2b, whh2b, bih2b, bhh2b,
                 w_out, b_out, T=T_FULL):
    NTOK = BL * T
    NTT = NTOK // 128
    bf = ml_dtypes.bfloat16

    # selector S[k, col]: k = gt*4 + cc*2 + hc ; col = gt*64 + cc*32 + hc*16 + b
    S = np.zeros((16, 256), np.float32)
    for gt in range(4):
        for cci in range(2):
            for hc in range(2):
                k = gt * 4 + cci * 2 + hc
                base = gt * 64 + cci * 32 + hc * 16
                S[k, base:base + 16] = 1.0

    common = {
        "emb": np.asarray(emb, np.float32).astype(bf),
        "ident128b": np.eye(128).astype(bf),
        "onescol": np.ones((1, 128), np.float32).astype(bf),
        "bsel": np.tile(S, (1, 2)).astype(bf),
        "woutT": np.ascontiguousarray(
            np.asarray(w_out, np.float32).T * 2.0).astype(bf),
        "bout": np.asarray(b_out, np.float32).reshape(1, TAGS).astype(bf),
    }
    brows = {}
    for cell, (wi, wh, bi, bh, hin) in {
        "1f": (wih1f, whh1f, bih1f, bhh1f, 1.0),
        "1b": (wih1b, whh1b, bih1b, bhh1b, 1.0),
        "2f": (wih2f, whh2f, bih2f, bhh2f, 2.0),
        "2b": (wih2b, whh2b, bih2b, bhh2b, 2.0),
    }.items():
        wihT, whhT, brow = _prep_cell_weights(
            np.asarray(wi, np.float32), np.asarray(wh, np.float32),
            np.asarray(bi, np.float32), np.asarray(bh, np.float32), hin)
        common[f"wih{cell}"] = wihT.astype(bf)
        common[f"whh{cell}"] = whhT.astype(bf)
        brows[cell] = brow
    # WbT[k, p] = beta_cell[gt*256 + hc*128 + p],  k = gt*4 + cc*2 + hc
    for layer, (cf, cb) in ((1, ("1f", "1b")), (2, ("2f", "2b"))):
        Wb = np.zeros((16, 128), np.float32)
        for gt in range(4):
            for cci, cell in enumerate((cf, cb)):
                for hc in range(2):
                    k = gt * 4 + cci * 2 + hc
                    Wb[k, :] = brows[cell][gt * 256 + hc * 128:
                                           gt * 256 + hc * 128 + 128]
        common[f"wb{layer}"] = Wb.astype(bf)

    sentence = np.asarray(sentence)
    in_maps = []
    for c in range(NCORES):
        sl = sentence[c * BL:(c + 1) * BL, :T]
        flat = np.ascontiguousarray(sl.T).reshape(NTOK)
        sent_in = np.ascontiguousarray(
            flat.reshape(NTT, 128).T.astype(np.int32))
        m = dict(common)
        m["sent"] = sent_in
        in_maps.append(m)
    return in_maps


def kernel(sentence, emb,
           wih1f, whh1f, bih1f, bhh1f,
           wih1b, whh1b, bih1b, bhh1b,
           wih2f, whh2f, bih2f, bhh2f,
           wih2b, whh2b, bih2b, bhh2b,
           w_out, b_out, _T=T_FULL):
    T = _T
    rn = get_runner(T)
    in_maps = make_in_maps(sentence, emb,
                           wih1f, whh1f, bih1f, bhh1f,
                           wih1b, whh1b, bih1b, bhh1b,
                           wih2f, whh2f, bih2f, bhh2f,
                           wih2b, whh2b, bih2b, bhh2b,
                           w_out, b_out, T=T)
    rn.put(in_maps)
    outs = rn.run()
    res = rn.results(outs)
    full = np.concatenate(
        [res[c]["out"].reshape(TAGS, T, BL).transpose(2, 1, 0)
         for c in range(NCORES)], axis=0)
    return full


# revision 39
# speedup vs baseline: 1.1538x; 1.1538x over previous
"""Bass/Trainium2 kernel for the BiLSTM tagger problem (transposed design).

Self-contained: builds an SPMD bass program (same program on all 8 cores,
data-parallel over the batch: 16 sentences/core), runs it via a bass2jax
shard_map runner, and gathers the full [128, 256, 50] output.

Layout: everything transposed — partition dim = feature dim, free dim =
(cell, h-chunk, batch).  Per core (Bl=16, T=256):

  E   : gather bf16 embeddings (indirect DMA, ends-inward tile order so L1
        can start immediately) + PE-transpose -> embT [128(E), 4096] bf16
  L1/L2 recurrence, fwd+bwd cells packed per step s (tf=s, tb=T-1-s):
        gatesT psum [128, 256] f32, col = gate(o,i,f,g)*64 + cc*32 + hc*16 + b
        bias  : 1 matmul  K=16 selector (WbT [16,128] @ S [16,256])
        pre   : L1: 16 mm N=16 (embT cols), L2: 64 mm N=16 (hist1 cols)
        hh    : 32 mm N=16 (lhsT = WhhT block [128,128], rhs = hT scratch)
        sgi   = sigmoid(gatesT[i,f,g])          (ACT; tanh via 2sig(2x)-1)
        sgo   = sigmoid(gatesT[o])              (ACT, off critical path)
        b     = sgi_f * c_prev                  (DVE tt)
        a'    = (sgi_g - 0.5) * sgi_i           (DVE stt)   [a = 2a']
        c     = 2*a' + b                        (DVE stt)
        tc'   = sigmoid(2c)                     (ACT, scale=2) [tanh(c)=2tc'-1]
        hT    = (tc' - 0.5) * sgo               (DVE stt -> scratch, feeds hh)
        hist_f/hist_b <- hT halves              (Pool copies, off the chain)
        hist stores h' = h/2; consumers (Whh, W2ih, wout) host-scaled x2.
  OUT : interleaved into the L2 loop as token tiles complete:
        per tile g: 8x(1+4) mm N=50 + Pool copies -> out [4096, 50] f32
"""

import os
import numpy as np
import ml_dtypes

B, T_FULL = 128, 256
VOCAB, EMB, HID, TAGS = 50000, 128, 256, 50
NCORES = 8
BL = B // NCORES            # 16 sentences per core
G4 = 4 * HID                # 1024
PSUM_BUFS = int(os.environ.get("K_PSUM_BUFS", "4"))
SG_BUFS = int(os.environ.get("K_SG_BUFS", "4"))


def _patched_tile_context(nc):
    """TileContext whose final drain splits sem waits across nops (this
    walrus build allows only one sync wait on control instructions)."""
    import concourse.tile as tile
    from concourse import mybir

    class PatchedTileContext(tile.TileContext):
        MAX_W = 1       # control insts (nop/drain) + PE (ldweights encoding)
        MAX_W_SOFT = int(os.environ.get("K_MAXW", "1"))  # other engines

        def _add_instruction(self, inst):
            si = inst.sync_info
            lim = self.MAX_W
            if inst.engine in (mybir.EngineType.PE, mybir.EngineType.SP):
                lim = self.MAX_W
            elif not isinstance(inst, (mybir.InstNoOp, mybir.InstDrain)):
                lim = self.MAX_W_SOFT
            if si is not None and si.on_wait and len(si.on_wait) > lim:
                waits = list(si.on_wait)
                si.on_wait = waits[-lim:]
                rest = waits[:-lim]
                while rest:
                    nop = mybir.InstNoOp(
                        name=self.nc.get_next_instruction_name(),
                        ins=[], outs=[])
                    nop.engine = inst.engine
                    nop.sync_info = mybir.SyncInfo(
                        on_wait=rest[:self.MAX_W], on_update=[])
                    rest = rest[self.MAX_W:]
                    super()._add_instruction(nop)
            super()._add_instruction(inst)

        def _drain_and_barrier(self, tick_clock, wait_clock):
            nop_inst = self.nc.sync.nop()
            wait_clock.add_sem_waits(
                nop_inst.ins, tile.ScopedClock({None: tick_clock.global_clock})
            )
            si = nop_inst.ins.sync_info
            waits = list(si.on_wait) if si is not None else []
            MAX_W = 1
            if len(waits) > MAX_W:
                si.on_wait = waits[:MAX_W]
                rest = waits[MAX_W:]
                while rest:
                    extra = self.nc.sync.nop()
                    extra.ins.sync_info = mybir.SyncInfo(
                        on_wait=rest[:MAX_W], on_update=[]
                    )
                    rest = rest[MAX_W:]
            self.nc.sync.drain()
            self.nc.all_engine_barrier()
            assert self.sems is not None
            popped = self.nc._tile_sem_poison_stack.pop()
            assert popped is self._sem_poison
            self.nc.clear_and_free_semaphores(list(self.sems.allocated().values()))
            self.nc.all_engine_barrier()

    return PatchedTileContext(nc)


def build_program(T=T_FULL):
    import concourse.bass as bass
    import concourse.mybir as mybir

    f32 = mybir.dt.float32
    i32 = mybir.dt.int32
    bf16 = mybir.dt.bfloat16
    SIG = mybir.ActivationFunctionType.Sigmoid
    MUL = mybir.AluOpType.mult
    ADD = mybir.AluOpType.add
    SUB = mybir.AluOpType.subtract

    NTOK = BL * T
    NTT = NTOK // 128       # token tiles (32)

    nc = bass.Bass()

    # ---------------- I/O ----------------
    sent = nc.dram_tensor("sent", [128, NTT], i32, kind="ExternalInput")
    emb_d = nc.dram_tensor("emb", [VOCAB, EMB], bf16, kind="ExternalInput")
    ident_d = nc.dram_tensor("ident128b", [128, 128], bf16, kind="ExternalInput")
    onescol_d = nc.dram_tensor("onescol", [1, 128], bf16, kind="ExternalInput")
    bsel_d = nc.dram_tensor("bsel", [16, 512], bf16, kind="ExternalInput")
    w_in = {}
    for cell in ("1f", "1b"):
        w_in[f"wih{cell}"] = nc.dram_tensor(f"wih{cell}", [EMB, G4], bf16,
                                            kind="ExternalInput")
    for cell in ("2f", "2b"):
        w_in[f"wih{cell}"] = nc.dram_tensor(f"wih{cell}", [2 * HID, G4], bf16,
                                            kind="ExternalInput")
    for cell in ("1f", "1b", "2f", "2b"):
        w_in[f"whh{cell}"] = nc.dram_tensor(f"whh{cell}", [HID, G4], bf16,
                                            kind="ExternalInput")
    wb_d = {1: nc.dram_tensor("wb1", [16, 128], bf16, kind="ExternalInput"),
            2: nc.dram_tensor("wb2", [16, 128], bf16, kind="ExternalInput")}
    wout_d = nc.dram_tensor("woutT", [2 * HID, TAGS], bf16, kind="ExternalInput")
    bout_d = nc.dram_tensor("bout", [1, TAGS], bf16, kind="ExternalInput")
    out_d = nc.dram_tensor("out", [TAGS, NTOK], f32, kind="ExternalOutput")

    tc = _patched_tile_context(nc)
    with tc:
        with tc.tile_pool(name="const", bufs=1) as cp:
            ident = cp.tile([128, 128], bf16)
            nc.sync.dma_start(ident[:], ident_d[:])
            onescol = cp.tile([1, 128], bf16)
            nc.sync.dma_start(onescol[:], onescol_d[:])
            bsel = cp.tile([16, 512], bf16)
            nc.sync.dma_start(bsel[:], bsel_d[:])
            NTT_ = BL * T // 128
            sidx = cp.tile([128, NTT_], i32, name="sidx")
            nc.sync.dma_start(sidx[:], sent[:, 0:NTT_])
            wb = {}
            for layer in (1, 2):
                wb[layer] = cp.tile([16, 128], bf16, tag=f"wb{layer}",
                                    name=f"swb{layer}")
                nc.sync.dma_start(wb[layer][:], wb_d[layer][:])
            # --- all LSTM weights, preloaded upfront; L1's first so its
            # first steps aren't stuck behind the (bigger) L2 loads ---
            wih1, whh = {}, {}
            for i, cell in enumerate(("1f", "1b")):
                wt = cp.tile([EMB, G4], bf16, tag=f"wih{cell}",
                             name=f"swih{cell}")
                nc.sync.dma_start(wt[:], w_in[f"wih{cell}"][:])
                wih1[i] = wt
            for cell in ("1f", "1b", "2f", "2b"):
                hh = []
                for k in range(2):
                    ht = cp.tile([128, G4], bf16, tag=f"whh{cell}{k}",
                                 name=f"swhh{cell}{k}")
                    nc.sync.dma_start(
                        ht[:], w_in[f"whh{cell}"][128 * k:128 * (k + 1), :])
                    hh.append(ht)
                whh[cell] = hh
            wih2 = {}
            for i, cell in enumerate(("2f", "2b")):
                ch = []
                for k in range(4):
                    wt = cp.tile([128, G4], bf16, tag=f"wih{cell}{k}",
                                 name=f"swih{cell}{k}")
                    nc.sync.dma_start(
                        wt[:], w_in[f"wih{cell}"][128 * k:128 * (k + 1), :])
                    ch.append(wt)
                wih2[i] = ch
            wout_ch = []
            for k in range(4):
                wt = cp.tile([128, TAGS], bf16, tag=f"wout{k}", name=f"swout{k}")
                nc.sync.dma_start(wt[:], wout_d[128 * k:128 * (k + 1), :])
                wout_ch.append(wt)
            bout = cp.tile([1, TAGS], bf16)
            nc.sync.dma_start(bout[:], bout_d[:])
            whh1 = {0: whh["1f"], 1: whh["1b"]}
            whh2 = {0: whh["2f"], 1: whh["2b"]}

            def lstm_layer(tc, layer, pre_pair, whhl, hT, T,
                           post_step=None, pre_step=None, rev_copy=None):
                """T steps (as T/2 two-step pairs), f+b cells packed.
                pre_pair(gpr, sp, cc, gc) issues the input-projection matmuls
                for one gate-chunk as [2-step, 16] blocks into the pair tile.
                Gate col layout per step: gate(o,i,f,g)*64 + cc*32 + hc*16 + b.
                h' is written straight into histf/histb (no scratch: slices
                are written once, so the writes carry no WAR waits);
                rev_copy(cc, t, s) mirrors them into reversed-order tiles."""
                with tc.tile_pool(name=f"l{layer}_work", bufs=SG_BUFS) as lp, \
                        tc.tile_pool(name=f"l{layer}_cp", bufs=3) as cpp, \
                        tc.tile_pool(name=f"l{layer}_psum", bufs=3,
                                     space="PSUM") as pp:
                    c_prev = None
                    for sp in range(T // 4):
                        gpp = pp.tile([128, 1024], f32, tag="gp", name="gp")
                        gpr = gpp.rearrange('p (t x) -> p t x', t=4)
                        # bias for all 4 steps (K=16 selector matmuls)
                        nc.tensor.matmul(gpp[:, 0:512], wb[layer][:], bsel[:],
                                         start=True, stop=False,
                                         skip_group_check=True)
                        nc.tensor.matmul(gpp[:, 512:1024], wb[layer][:],
                                         bsel[:], start=True, stop=False,
                                         skip_group_check=True)
                        # input projections for the 4-step block
                        for cc in (0, 1):
                            for gc in range(8):
                                pre_pair(gpr, sp, cc, gc)
                        for shalf in range(4):
                            s = 4 * sp + shalf
                            if pre_step is not None:
                                pre_step(s)
                            ts_ = {0: s, 1: T - 1 - s}
                            gp = gpp[:, shalf * 256:shalf * 256 + 256]
                            # hh: both cells read the previous step's block
                            # of the write-once hT history
                            if s > 0:
                                hb = (s - 1) * 64
                                for cc in (0, 1):
                                    for gc in range(8):
                                        gt, hc = gc // 2, gc % 2
                                        cbase = gt * 64 + cc * 32 + hc * 16
                                        csl = slice(cbase, cbase + 16)
                                        for kc in range(2):
                                            nc.tensor.matmul(
                                                gp[:, csl],
                                                whhl[cc][kc][:, gc * 128:
                                                             (gc + 1) * 128],
                                                hT[:, hb + cc * 32 + kc * 16:
                                                    hb + cc * 32 + kc * 16 + 16],
                                                start=False, stop=(kc == 1),
                                                skip_group_check=True)
                            # ---- pointwise ----
                            sgi = lp.tile([128, 192], f32, tag="sgi", name="sgi")
                            nc.scalar.activation(sgi[:], gp[:, 64:256], SIG)
                            sgo = lp.tile([128, 64], f32, tag="sgo", name="sgo")
                            nc.scalar.activation(sgo[:], gp[:, 0:64], SIG)
                            ap_t = lp.tile([128, 64], f32, tag="ap")
                            if s == 0:
                                nc.vector.scalar_tensor_tensor(
                                    ap_t[:], sgi[:, 128:192], 0.5, sgi[:, 0:64],
                                    SUB, MUL)
                                c_new = cpp.tile([128, 64], f32, tag="c",
                                                 name="c")
                                nc.vector.tensor_scalar(c_new[:], ap_t[:], 2.0,
                                                        None, MUL)
                            else:
                                b_t = lp.tile([128, 64], f32, tag="bb")
                                nc.vector.tensor_tensor(
                                    b_t[:], sgi[:, 64:128], c_prev[:], MUL)
                                nc.vector.scalar_tensor_tensor(
                                    ap_t[:], sgi[:, 128:192], 0.5, sgi[:, 0:64],
                                    SUB, MUL)
                                c_new = cpp.tile([128, 64], f32, tag="c",
                                                 name="c")
                                nc.vector.scalar_tensor_tensor(
                                    c_new[:], ap_t[:], 2.0, b_t[:], MUL, ADD)
                            c_prev = c_new
                            tcp = lp.tile([128, 64], f32, tag="tc")
                            nc.scalar.activation(tcp[:], c_new[:], SIG,
                                                 scale=2.0)
                            nc.vector.scalar_tensor_tensor(
                                hT[:, s * 64:s * 64 + 64],
                                tcp[:], 0.5, sgo[:], SUB, MUL)
                            if rev_copy is not None:
                                rev_copy(s)
                            if post_step is not None:
                                post_step(s)

            # ================= hist + embT =================
            # hT layout: step-block col = s*64 + cc*32 + hc*16 + b (write-once;
            # f-half by token t=s, b-half by token T-1-s).  Mirrors h1fr
            # (h1f reversed) and h1btok (h1b token-ordered) cover the two
            # descending access patterns L2's blocked projections need.
            with tc.tile_pool(name="hist", bufs=1) as hp:
                hT1 = hp.tile([128, 64 * T], bf16, tag="hT1", name="hT1")
                h1fr = hp.tile([128, 32 * T], bf16, tag="h1fr", name="h1fr")
                h1btok = hp.tile([128, 32 * T], bf16, tag="h1btok",
                                 name="h1btok")
                hT2 = hp.tile([128, 64 * T], bf16, tag="hT2", name="hT2")

                with tc.tile_pool(name="l1_fix", bufs=1) as p_l1:
                    embT = p_l1.tile([128, NTOK], bf16)
                    embTr = p_l1.tile([128, NTOK], bf16, name="embTr")
                    # gather ends-inward, interleaved with the L1 steps, so
                    # L1 (f from tile 0, b from tile 31) starts immediately
                    # and the middle tiles stream in ahead of consumption
                    with tc.tile_pool(name="embp", bufs=4) as ep, \
                            tc.tile_pool(name="embpp", bufs=2,
                                         space="PSUM") as epp:
                        def gather(g):
                            et = ep.tile([128, EMB], bf16, tag="et")
                            nc.gpsimd.indirect_dma_start(
                                out=et[:], out_offset=None,
                                in_=emb_d[:],
                                in_offset=bass.IndirectOffsetOnAxis(
                                    ap=sidx[:, g:g + 1], axis=0),
                            )
                            etp = epp.tile([128, EMB], bf16, tag="etp")
                            nc.tensor.transpose(etp[:], et[:], ident[:])
                            nc.vector.tensor_copy(
                                embT[:, 128 * g:128 * (g + 1)], etp[:])
                            # mirrored copy: token t -> col (T-1-t)*16
                            # (from embT: gpsimd cannot read PSUM)
                            for tau in range(8):
                                t = g * 8 + tau
                                nc.gpsimd.tensor_copy(
                                    embTr[:, (T - 1 - t) * 16:
                                          (T - 1 - t) * 16 + 16],
                                    embT[:, t * 16:t * 16 + 16])

                        for g in (0, NTT - 1, 1, NTT - 2):
                            gather(g)

                        def pre_step1(s):
                            if s % 8 == 0 and s // 8 + 2 <= NTT // 2 - 1:
                                k = s // 8
                                gather(k + 2)
                                gather(NTT - 3 - k)

                        def pre1(gpr, sp, cc, gc):
                            gt, hc = gc // 2, gc % 2
                            off = gt * 64 + cc * 32 + hc * 16
                            src = embT if cc == 0 else embTr
                            nc.tensor.matmul(
                                gpr[:, :, off:off + 16],
                                wih1[cc][:, gc * 128:(gc + 1) * 128],
                                src[:, sp * 64:sp * 64 + 64],
                                start=False, stop=False,
                                skip_group_check=True)

                        def rev1(s):
                            # h1fr[(T-1-s)*32] <- f-half of step s;
                            # h1btok[(T-1-s)*32] <- b-half (token T-1-s)
                            nc.gpsimd.tensor_copy(
                                h1fr[:, (T - 1 - s) * 32:(T - 1 - s) * 32 + 32],
                                hT1[:, s * 64:s * 64 + 32])
                            nc.gpsimd.tensor_copy(
                                h1btok[:, (T - 1 - s) * 32:
                                       (T - 1 - s) * 32 + 32],
                                hT1[:, s * 64 + 32:s * 64 + 64])

                        # inside the gather pools: no drain barrier, so L1
                        # starts as soon as the edge tiles land
                        lstm_layer(tc, 1, pre1, whh1, hT1, T,
                                   pre_step=pre_step1, rev_copy=rev1)
                # p_l1 closed: embT freed

                # ================= L2 (+ interleaved OUT) =================
                # x2(token t) sources for the blocked projections (all read
                # ascending): f-cell: (hT1 f-half, h1btok); b-cell (step
                # s = T-1-t ascending): (h1fr, hT1 b-half)
                hT1v = hT1.rearrange('p (t x) -> p t x', t=T)
                h1frv = h1fr.rearrange('p (t x) -> p t x', t=T)
                h1btokv = h1btok.rearrange('p (t x) -> p t x', t=T)

                def pre2(gpr, sp, cc, gc):
                    gt, hc = gc // 2, gc % 2
                    off = gt * 64 + cc * 32 + hc * 16
                    for kc in range(4):
                        ko = (kc % 2) * 16
                        if cc == 0:
                            src, so = ((hT1v, ko) if kc < 2
                                       else (h1btokv, ko))
                        else:
                            src, so = ((h1frv, ko) if kc < 2
                                       else (hT1v, 32 + ko))
                        nc.tensor.matmul(
                            gpr[:, :, off:off + 16],
                            wih2[cc][kc][:, gc * 128:(gc + 1) * 128],
                            src[:, 4 * sp:4 * sp + 4, so:so + 16],
                            start=False, stop=False, skip_group_check=True)

                # OUT tile g is ready after L2 step max(8g+7, 255-8g)
                ready = {}
                for g in range(NTT):
                    ready.setdefault(max(8 * g + 7, T - 1 - 8 * g), []).append(g)

                with tc.tile_pool(name="outw", bufs=3) as ow, \
                        tc.tile_pool(name="outp", bufs=2, space="PSUM") as op:

                    def out_tile(g):
                        # transposed: ps[tag, token] so per-step results land
                        # in free-dim column slices (no partition-base limits)
                        ps = op.tile([TAGS, 128], f32, tag="ops")
                        nc.tensor.matmul(ps[:], bout[:1, :], onescol[:1, :],
                                         start=True, stop=False,
                                         skip_group_check=True)
                        for tau in range(8):
                            t = g * 8 + tau
                            for k in range(4):
                                cc, hc = k // 2, k % 2
                                if cc == 0:
                                    col = t * 64 + hc * 16
                                else:
                                    col = (T - 1 - t) * 64 + 32 + hc * 16
                                nc.tensor.matmul(
                                    ps[:, tau * 16:tau * 16 + 16],
                                    wout_ch[k][:],
                                    hT2[:, col:col + 16],
                                    start=False, stop=(k == 3),
                                    skip_group_check=True)
                        sb = ow.tile([TAGS, 128], f32, tag="osb")
                        nc.vector.tensor_copy(sb[:], ps[:])
                        nc.sync.dma_start(out_d[:, 128 * g:128 * (g + 1)],
                                          sb[:])

                    def post_step(s):
                        for g in ready.get(s, ()):
                            out_tile(g)

                    lstm_layer(tc, 2, pre2, whh2, hT2, T,
                               post_step=post_step)

    return nc


def _prep_cell_weights(wih, whh, bih, bhh, h_in_scale):
    """Gate perm i,f,g,o -> o,i,f,g; scale g rows x2 (tanh(x)=2*sig(2x)-1);
    whh rows x2 and wih rows x h_in_scale compensate the h'=h/2 storage.
    Returns (wihT, whhT, brow) f32 with transposed [Din, 4H] layout."""
    H = HID
    idx = np.concatenate([np.arange(3 * H, 4 * H),      # o
                          np.arange(0, H),              # i
                          np.arange(H, 2 * H),          # f
                          np.arange(2 * H, 3 * H)])     # g
    gscale = np.ones((4 * H, 1), np.float32)
    gscale[3 * H:4 * H] = 2.0
    wih_p = wih[idx] * gscale * h_in_scale
    whh_p = whh[idx] * gscale * 2.0
    b_p = (bih + bhh)[idx] * gscale[:, 0]
    return (np.ascontiguousarray(wih_p.T, np.float32),
            np.ascontiguousarray(whh_p.T, np.float32),
            np.ascontiguousarray(b_p, np.float32))


class Runner:
    """Build the SPMD program once; execute repeatedly on device-resident
    inputs (for clean timing, no donation so buffers are reusable)."""

    def __init__(self, nc, n_cores=NCORES):
        import jax
        import numpy as _np
        from jax.sharding import Mesh, PartitionSpec
        from jax.experimental.shard_map import shard_map
        import concourse.mybir as mybir
        from concourse import bass2jax as b2j

        b2j.install_neuronx_cc_hook()
        self.jax = jax
        self.nc = nc
        self.n_cores = n_cores
        partition_name = (nc.partition_id_tensor.name
                          if nc.partition_id_tensor else None)
        in_names, out_names, out_avals, zero_outs = [], [], [], []
        for alloc in nc.m.functions[0].allocations:
            if not isinstance(alloc, mybir.MemoryLocationSet):
                continue
            name = alloc.memorylocations[0].name
            if alloc.kind == "ExternalInput":
                if name != partition_name:
                    in_names.append(name)
            elif alloc.kind == "ExternalOutput":
                out_names.append(name)
                shape = tuple(alloc.tensor_shape)
                dtype = mybir.dt.np(alloc.dtype)
                out_avals.append(jax.core.ShapedArray(shape, dtype))
                zero_outs.append(_np.zeros(shape, dtype))
        self.n_params = len(in_names)
        self.in_names = list(in_names)
        self.out_names = list(out_names)
        self.out_avals = out_avals
        self.zero_outs = zero_outs
        all_in = in_names + out_names
        if partition_name is not None:
            all_in.append(partition_name)

        def _body(*args):
            operands = list(args)
            if partition_name is not None:
                operands.append(b2j.partition_id_tensor())
            outs = b2j._bass_exec_p.bind(
                *operands,
                out_avals=tuple(out_avals),
                in_names=tuple(all_in),
                out_names=tuple(out_names),
                lowering_input_output_aliases=(),
                sim_require_finite=True,
                sim_require_nnan=True,
                nc=nc,
            )
            return tuple(outs)

        devices = jax.devices()[:n_cores]
        self.mesh = Mesh(_np.asarray(devices), ("core",))
        in_specs = (PartitionSpec("core"),) * (self.n_params + len(out_names))
        out_specs = (PartitionSpec("core"),) * len(out_names)
        self.sharded = jax.jit(shard_map(_body, mesh=self.mesh,
                                         in_specs=in_specs,
                                         out_specs=out_specs, check_rep=False),
                               keep_unused=True)
        self.dev_args = None

    def put(self, in_maps):
        """Upload per-core input maps as device-sharded global arrays."""
        import numpy as _np
        from jax.sharding import NamedSharding, PartitionSpec
        jax = self.jax
        sh = NamedSharding(self.mesh, PartitionSpec("core"))
        args = []
        for name in self.in_names:
            g = _np.concatenate([_np.asarray(m[name]) for m in in_maps], axis=0)
            args.append(jax.device_put(g, sh))
        for z in self.zero_outs:
            g = _np.zeros((self.n_cores * z.shape[0],) + z.shape[1:], z.dtype)
            args.append(jax.device_put(g, sh))
        self.dev_args = args

    def run(self):
        outs = self.sharded(*self.dev_args)
        self.jax.block_until_ready(outs)
        return outs

    def results(self, outs):
        import numpy as _np
        res = []
        for c in range(self.n_cores):
            res.append({name: _np.asarray(outs[i]).reshape(
                (self.n_cores,) + self.out_avals[i].shape)[c]
                for i, name in enumerate(self.out_names)})
        return res

    def time_exec(self, iters=10):
        import time as _time
        self.run()  # warm
        best = float("inf")
        for _ in range(iters):
            t0 = _time.perf_counter()
            self.run()
            best = min(best, _time.perf_counter() - t0)
        return best


_RUNNERS = {}


def get_runner(T=T_FULL):
    if T not in _RUNNERS:
        _RUNNERS[T] = Runner(build_program(T))
    return _RUNNERS[T]


def make_in_maps(sentence, emb,
                 wih1f, whh1f, bih1f, bhh1f,
                 wih1b, whh1b, bih1b, bhh1b,
                 wih2f, whh2f, bih2f, bhh2f,
                 wih2b, whh2b, bih2b, bhh2b,
                 w_out, b_out, T=T_FULL):
    NTOK = BL * T
    NTT = NTOK // 128
    bf = ml_dtypes.bfloat16

    # selector S[k, col]: k = gt*4 + cc*2 + hc ; col = gt*64 + cc*32 + hc*16 + b
    S = np.zeros((16, 256), np.float32)
    for gt in range(4):
        for cci in range(2):
            for hc in range(2):
                k = gt * 4 + cci * 2 + hc
                base = gt * 64 + cci * 32 + hc * 16
                S[k, base:base + 16] = 1.0

    common = {
        "emb": np.asarray(emb, np.float32).astype(bf),
        "ident128b": np.eye(128).astype(bf),
        "onescol": np.ones((1, 128), np.float32).astype(bf),
        "bsel": np.tile(S, (1, 2)).astype(bf),
        "woutT": np.ascontiguousarray(
            np.asarray(w_out, np.float32).T * 2.0).astype(bf),
        "bout": np.asarray(b_out, np.float32).reshape(1, TAGS).astype(bf),
    }
    brows = {}
    for cell, (wi, wh, bi, bh, hin) in {
        "1f": (wih1f, whh1f, bih1f, bhh1f, 1.0),
        "1b": (wih1b, whh1b, bih1b, bhh1b, 1.0),
        "2f": (wih2f, whh2f, bih2f, bhh2f, 2.0),
        "2b": (wih2b, whh2b, bih2b, bhh2b, 2.0),
    }.items():
        wihT, whhT, brow = _prep_cell_weights(
            np.asarray(wi, np.float32), np.asarray(wh, np.float32),
            np.asarray(bi, np.float32), np.asarray(bh, np.float32), hin)
        common[f"wih{cell}"] = wihT.astype(bf)
        common[f"whh{cell}"] = whhT.astype(bf)
        brows[cell] = brow
    # WbT[k, p] = beta_cell[gt*256 + hc*128 + p],  k = gt*4 + cc*2 + hc
    for layer, (cf, cb) in ((1, ("1f", "1b")), (2, ("2f", "2b"))):
        Wb = np.zeros((16, 128), np.float32)
        for gt in range(4):
            for cci, cell in enumerate((cf, cb)):
                for hc in range(2):
                    k = gt * 4 + cci * 2 + hc
                    Wb[k, :] = brows[cell][gt * 256 + hc * 128:
                                           gt * 256 + hc * 128 + 128]
        common[f"wb{layer}"] = Wb.astype(bf)

    sentence = np.asarray(sentence)
    in_maps = []
    for c in range(NCORES):
        sl = sentence[c * BL:(c + 1) * BL, :T]
        flat = np.ascontiguousarray(sl.T).reshape(NTOK)
        sent_in = np.ascontiguousarray(
            flat.reshape(NTT, 128).T.astype(np.int32))
        m = dict(common)
        m["sent"] = sent_in
        in_maps.append(m)
    return in_maps


def kernel(sentence, emb,
           wih1f, whh1f, bih1f, bhh1f,
           wih1b, whh1b, bih1b, bhh1b,
           wih2f, whh2f, bih2f, bhh2f,
           wih2b, whh2b, bih2b, bhh2b,
           w_out, b_out, _T=T_FULL):
    T = _T
    rn = get_runner(T)
    in_maps = make_in_maps(sentence, emb,
                           wih1f, whh1f, bih1f, bhh1f,
                           wih1b, whh1b, bih1b, bhh1b,
                           wih2f, whh2f, bih2f, bhh2f,
                           wih2b, whh2b, bih2b, bhh2b,
                           w_out, b_out, T=T)
    rn.put(in_maps)
    outs = rn.run()
    res = rn.results(outs)
    full = np.concatenate(
        [res[c]["out"].reshape(TAGS, T, BL).transpose(2, 1, 0)
         for c in range(NCORES)], axis=0)
    return full
